# revision 5
# baseline (speedup 1.0000x reference)
"""Trainium2 Bass kernel for EnhancedGNNModelWithMLP (5x SAGEConv+GraphNorm+ReLU, 5 MLP heads).

Sharding: 8 cores, nodes partitioned contiguously (62976 padded per core), edges
partitioned by destination owner and sorted by destination. Per layer on device:
indirect-DMA gather of source rows from a bf16 replicated copy of h, segment-sum
via one-hot-mask matmuls (masks from DVE is_equal vs constant iota), mean via
host-precomputed 1/deg, Wl/Wr applied feature-major, GraphNorm stats accumulated
per-core via ACT accum_out + tiny AllReduce, normalize+ReLU pass emitting fp32
h'^T (next root input) and bf16 node-major h' AllGathered for the next gather.
MLP heads fused into layer-5's normalize pass.

Self-contained: hardcodes all shapes. kernel(**inputs) takes full inputs and
returns the tuple of 5 head outputs.
"""
import sys
import types

import numpy as np
import ml_dtypes

NCORE = 8
N_REAL = 500022
E = 2500000
D = 128
TILE = 128
TPST = 4                    # tiles per supertile
ST_C = 123                  # supertiles per core
NT_C = ST_C * TPST          # 492 tiles per core
NPC = NT_C * TILE           # 62976 nodes per core
NPAD = NCORE * NPC          # 503808
MLP_H = 256
OUTS = (("age", 7), ("sex", 2), ("ethnicity", 5), ("religion", 4), ("marital", 4))
OC_TOT = 22
EPS = 1e-5

last_exec_time_ns = None


def _register_ntff_hook():
    try:
        from trn_agent_boot.trn_boot import _ntff_profile_via_ctypes
        hook = _ntff_profile_via_ctypes("/opt/axon/libaxon_pjrt.so")
        import antenv
        mod = types.ModuleType("antenv.axon_hooks")
        mod.get_axon_ntff_profile_hook = lambda: hook
        sys.modules["antenv.axon_hooks"] = mod
        antenv.axon_hooks = mod
        return hook is not None
    except Exception:
        return False


# ---------------------------------------------------------------- host prep

def _prep(x, edge_index, params):
    """Shard + sort edges by destination, build per-core device input arrays."""
    x = np.asarray(x, np.float32)
    src_o = np.asarray(edge_index[0], np.int64)
    dst_o = np.asarray(edge_index[1], np.int64)

    counts = np.array([62503] * 6 + [62502] * 2)
    assert counts.sum() == N_REAL
    starts = np.concatenate([[0], np.cumsum(counts)])
    pad_of = np.empty(N_REAL, np.int64)
    for c in range(NCORE):
        pad_of[starts[c]:starts[c + 1]] = c * NPC + np.arange(counts[c])

    srcp = pad_of[src_o]
    dstp = pad_of[dst_o]
    core_of = dstp // NPC

    per_core = []
    Gt_all = np.zeros((NCORE, NT_C), np.int64)
    for c in range(NCORE):
        m = core_of == c
        s_p = srcp[m]
        s_o = src_o[m]
        d_loc = dstp[m] - c * NPC
        order = np.argsort(d_loc, kind="stable")
        s_p, s_o, d_loc = s_p[order], s_o[order], d_loc[order]
        tile_id = d_loc // TILE
        tc = np.bincount(tile_id, minlength=NT_C)
        Gt_all[c] = np.maximum(1, -(-tc // TILE))
        per_core.append((s_p, s_o, d_loc, tile_id, tc))

    Gt = Gt_all.max(axis=0)               # groups per tile, uniform across cores
    goff = np.concatenate([[0], np.cumsum(Gt)])
    G_total = int(goff[-1])

    # static per-supertile structure for the builder
    n_st = [int(Gt[st * TPST:(st + 1) * TPST].sum()) for st in range(ST_C)]
    wsplit = [[int(g) for g in Gt[st * TPST:(st + 1) * TPST]] for st in range(ST_C)]
    g0_st = [int(goff[st * TPST]) for st in range(ST_C)]

    in_maps = []
    x_flat = x[:, 0]
    for c in range(NCORE):
        s_p, s_o, d_loc, tile_id, tc = per_core[c]
        tstart = np.concatenate([[0], np.cumsum(tc)])
        pos = goff[tile_id] * TILE + (np.arange(len(s_p)) - tstart[tile_id])
        flat_src = np.zeros(G_total * TILE, np.int32)
        flat_dst = np.full(G_total * TILE, 255.0, np.float32)
        flat_xg = np.zeros(G_total * TILE, np.float32)
        flat_src[pos] = s_p.astype(np.int32)
        flat_dst[pos] = (d_loc % TILE).astype(np.float32)
        flat_xg[pos] = x_flat[s_o]
        srcT = np.ascontiguousarray(flat_src.reshape(G_total, TILE).T)
        dstT = np.ascontiguousarray(flat_dst.reshape(G_total, TILE).T)
        xgT = np.ascontiguousarray(flat_xg.reshape(G_total, TILE).T)

        cnt = np.bincount(d_loc, minlength=NPC).astype(np.float32)
        cnt_inv = (1.0 / np.maximum(cnt, 1.0)).astype(np.float32)

        x_own = np.zeros(NPC, np.float32)
        nreal = counts[c]
        x_own[:nreal] = x_flat[starts[c]:starts[c + 1]]

        smask = np.zeros((1, NPC), np.float32)
        smask[0, :nreal] = 1.0

        im = {
            "srcT": srcT,
            "dstT": dstT.astype(ml_dtypes.bfloat16),
            "dstTf": dstT,                       # fp32 copy for layer 1 masks
            "xgT": xgT,
            "cntinv": cnt_inv[None, :],
            "x_own": x_own[None, :],
            "smask": smask,
        }
        in_maps.append(im)

    # constants shared by all cores
    iota_f = np.ascontiguousarray(
        np.broadcast_to(np.arange(TILE, dtype=np.float32), (TILE, TILE)))
    iota_b = iota_f.astype(ml_dtypes.bfloat16)
    ident = np.eye(TILE, dtype=np.float32)

    consts = {"iota_f": iota_f, "iota_b": np.ascontiguousarray(iota_b),
              "ident": ident, "invn": np.full((128, 1), 1.0 / N_REAL, np.float32)}

    convs = params["convs"]
    norms = params["norms"]
    consts["wl1"] = np.asarray(convs[0]["Wl"], np.float32).reshape(1, D)
    consts["wr1"] = np.asarray(convs[0]["Wr"], np.float32).reshape(1, D)
    for li in range(1, 5):
        consts[f"wl{li + 1}"] = np.asarray(convs[li]["Wl"], np.float32)
        consts[f"wr{li + 1}"] = np.asarray(convs[li]["Wr"], np.float32)
    for li in range(5):
        consts[f"bias{li + 1}"] = np.asarray(convs[li]["bl"], np.float32).reshape(D, 1)
        g = np.asarray(norms[li]["gamma"], np.float32)
        b = np.asarray(norms[li]["beta"], np.float32)
        a = np.asarray(norms[li]["alpha"], np.float32)
        consts[f"gamma{li + 1}"] = g.reshape(D, 1)
        consts[f"beta{li + 1}"] = b.reshape(D, 1)
        consts[f"alpha{li + 1}"] = a.reshape(D, 1)
        consts[f"calpha{li + 1}"] = (2 * a - a * a).reshape(D, 1)
    for hi, (name, oc) in enumerate(OUTS):
        (W1, b1), (W2, b2), (W3, b3) = params["mlps"][name]
        W1 = np.asarray(W1, np.float32); W2 = np.asarray(W2, np.float32)
        W3 = np.asarray(W3, np.float32)
        consts[f"h{hi}w1a"] = np.ascontiguousarray(W1[:, :128])
        consts[f"h{hi}w1b"] = np.ascontiguousarray(W1[:, 128:])
        consts[f"h{hi}w2a"] = np.ascontiguousarray(W2[:128, :])
        consts[f"h{hi}w2b"] = np.ascontiguousarray(W2[128:, :])
        consts[f"h{hi}w3"] = np.ascontiguousarray(W3)
        b1 = np.asarray(b1, np.float32)
        consts[f"h{hi}b1a"] = b1[:128].reshape(128, 1)
        consts[f"h{hi}b1b"] = b1[128:].reshape(128, 1)
        consts[f"h{hi}b2"] = np.asarray(b2, np.float32).reshape(128, 1)
        consts[f"h{hi}b3"] = np.asarray(b3, np.float32).reshape(oc, 1)

    for im in in_maps:
        im.update(consts)
    return in_maps, n_st, wsplit, g0_st, G_total, counts, starts


# ---------------------------------------------------------------- builder

def _build(n_st, wsplit, g0_st, G_total):
    import concourse.bass as bass
    import concourse.bacc as bacc
    import concourse.mybir as mybir
    import concourse.tile as tile

    f32 = mybir.dt.float32
    bf16 = mybir.dt.bfloat16
    i32 = mybir.dt.int32
    AF = mybir.ActivationFunctionType
    ALU = mybir.AluOpType

    nc = bacc.Bacc("TRN2", target_bir_lowering=False, debug=False, num_devices=NCORE)

    P = nc.declare_dram_parameter
    srcT = P("srcT", [TILE, G_total], i32, isOutput=False)
    dstT = P("dstT", [TILE, G_total], bf16, isOutput=False)
    dstTf = P("dstTf", [TILE, G_total], f32, isOutput=False)
    xgT = P("xgT", [TILE, G_total], f32, isOutput=False)
    cntinv = P("cntinv", [1, NPC], f32, isOutput=False)
    x_own = P("x_own", [1, NPC], f32, isOutput=False)
    smask = P("smask", [1, NPC], f32, isOutput=False)
    iota_f = P("iota_f", [TILE, TILE], f32, isOutput=False)
    iota_b = P("iota_b", [TILE, TILE], bf16, isOutput=False)
    ident = P("ident", [TILE, TILE], f32, isOutput=False)
    invn = P("invn", [128, 1], f32, isOutput=False)
    wl = {1: P("wl1", [1, D], f32, isOutput=False)}
    wr = {1: P("wr1", [1, D], f32, isOutput=False)}
    for li in range(2, 6):
        wl[li] = P(f"wl{li}", [D, D], f32, isOutput=False)
        wr[li] = P(f"wr{li}", [D, D], f32, isOutput=False)
    bias = {li: P(f"bias{li}", [D, 1], f32, isOutput=False) for li in range(1, 6)}
    gamma = {li: P(f"gamma{li}", [D, 1], f32, isOutput=False) for li in range(1, 6)}
    beta = {li: P(f"beta{li}", [D, 1], f32, isOutput=False) for li in range(1, 6)}
    alpha = {li: P(f"alpha{li}", [D, 1], f32, isOutput=False) for li in range(1, 6)}
    calpha = {li: P(f"calpha{li}", [D, 1], f32, isOutput=False) for li in range(1, 6)}
    hw = {}
    for hi, (_, oc) in enumerate(OUTS):
        for k in ("w1a", "w1b", "w2a", "w2b"):
            hw[(hi, k)] = P(f"h{hi}{k}", [128, 128], f32, isOutput=False)
        hw[(hi, "w3")] = P(f"h{hi}w3", [128, oc], f32, isOutput=False)
        for k in ("b1a", "b1b", "b2"):
            hw[(hi, k)] = P(f"h{hi}{k}", [128, 1], f32, isOutput=False)
        hw[(hi, "b3")] = P(f"h{hi}b3", [oc, 1], f32, isOutput=False)

    out_heads = P("out_heads", [NPC, OC_TOT], f32, isOutput=True)

    with tile.TileContext(nc) as tc:
        with (
            tc.tile_pool(name="const", bufs=1) as cp,
            tc.tile_pool(name="sb", bufs=2) as sb,
            tc.tile_pool(name="ps", bufs=2, space="PSUM") as ps,
            tc.tile_pool(name="dr", bufs=1, space="DRAM") as dr,
        ):
            # ---- DRAM working buffers
            h_reps = {l: dr.tile([NPAD, D], bf16, addr_space="Shared", name=f"h_rep{l}")
                      for l in range(1, 5)}
            ag_in = dr.tile([NPC, D], bf16, name="ag_in")
            hT_a = dr.tile([NT_C, TILE, TILE], f32, name="hT_a")
            hT_b = dr.tile([NT_C, TILE, TILE], f32, name="hT_b")
            zT_d = dr.tile([NT_C, TILE, TILE], f32, name="zT_d")
            ar_in = dr.tile([128, 2], f32, name="ar_in")
            ar_outs = {l: dr.tile([128, 2], f32, addr_space="Shared", name=f"ar_out{l}")
                       for l in range(1, 6)}

            # ---- constants to SBUF
            def cload(ap, shape, dtype, name):
                t = cp.tile(shape, dtype, name=name)
                nc.sync.dma_start(out=t[:], in_=ap[:])
                return t

            iota_f_t = cload(iota_f, [TILE, TILE], f32, "iota_f_t")
            iota_b_t = cload(iota_b, [TILE, TILE], bf16, "iota_b_t")
            ident_t = cload(ident, [TILE, TILE], f32, "ident_t")
            invn_t = cload(invn, [128, 1], f32, "invn_t")
            wl_t = {li: cload(wl[li], list(wl[li].shape), f32, f"wl_t{li}") for li in wl}
            wr_t = {li: cload(wr[li], list(wr[li].shape), f32, f"wr_t{li}") for li in wr}
            bias_t = {li: cload(bias[li], [D, 1], f32, f"bias_t{li}") for li in bias}
            gamma_t = {li: cload(gamma[li], [D, 1], f32, f"gamma_t{li}") for li in gamma}
            beta_t = {li: cload(beta[li], [D, 1], f32, f"beta_t{li}") for li in beta}
            alpha_t = {li: cload(alpha[li], [D, 1], f32, f"alpha_t{li}") for li in alpha}
            calpha_t = {li: cload(calpha[li], [D, 1], f32, f"calpha_t{li}") for li in calpha}
            hw_t = {}
            for hi, (_, oc) in enumerate(OUTS):
                for k in ("w1a", "w1b", "w2a", "w2b"):
                    hw_t[(hi, k)] = cload(hw[(hi, k)], [128, 128], f32, f"h{hi}{k}_t")
                hw_t[(hi, "w3")] = cload(hw[(hi, "w3")], [128, oc], f32, f"h{hi}w3_t")
                for k in ("b1a", "b1b", "b2"):
                    hw_t[(hi, k)] = cload(hw[(hi, k)], [128, 1], f32, f"h{hi}{k}_t")
                hw_t[(hi, "b3")] = cload(hw[(hi, "b3")], [oc, 1], f32, f"h{hi}b3_t")

            stats1 = cp.tile([128, 128], f32, name="stats1")
            stats2 = cp.tile([128, 128], f32, name="stats2")

            # =========================================================
            def pass_a(layer):
                hT_prev = hT_a if layer % 2 == 0 else hT_b
                nc.vector.memset(stats1[:], 0.0)
                nc.vector.memset(stats2[:], 0.0)
                for st in range(ST_C):
                    g0, n = g0_st[st], n_st[st]
                    c0 = st * TPST * TILE  # first local dst node of supertile

                    if layer == 1:
                        vals = sb.tile([TILE, n], f32, tag="vals1", bufs=3,
                                       name=f"v{layer}_{st}")
                        nc.sync.dma_start(out=vals[:], in_=xgT[:, g0:g0 + n])
                        dst_l = sb.tile([TILE, n], f32, tag="dstl1", bufs=3,
                                        name=f"dl{layer}_{st}")
                        nc.sync.dma_start(out=dst_l[:], in_=dstTf[:, g0:g0 + n])
                        agg_ps = ps.tile([1, 512], f32, tag="agg", bufs=2,
                                         name=f"agg{layer}_{st}")
                    else:
                        idx_t = sb.tile([TILE, n], i32, tag="idx", bufs=3,
                                        name=f"ix{layer}_{st}")
                        nc.sync.dma_start(out=idx_t[:], in_=srcT[:, g0:g0 + n])
                        dst_l = sb.tile([TILE, n], bf16, tag="dstl", bufs=3,
                                        name=f"dl{layer}_{st}")
                        nc.sync.dma_start(out=dst_l[:], in_=dstT[:, g0:g0 + n])
                        agg_ps = ps.tile([128, 512], f32, tag="agg", bufs=2,
                                         name=f"agg{layer}_{st}")

                    cnt_t = sb.tile([128, 512], f32, tag="cnt", bufs=2,
                                    name=f"cn{layer}_{st}")
                    nc.sync.dma_start(
                        out=cnt_t[:],
                        in_=cntinv[0:1, c0:c0 + 512].to_broadcast([128, 512]))

                    j = 0
                    for w in range(TPST):
                        gw = wsplit[st][w]
                        for k in range(gw):
                            if layer == 1:
                                sel = sb.tile([TILE, TILE], f32, tag="sel1", bufs=4,
                                              name=f"s{layer}_{st}_{j}")
                                nc.vector.tensor_tensor(
                                    out=sel[:],
                                    in0=dst_l[:, j:j + 1].to_broadcast([TILE, TILE]),
                                    in1=iota_f_t[:], op=ALU.is_equal)
                                nc.tensor.matmul(
                                    out=agg_ps[0:1, w * TILE:(w + 1) * TILE],
                                    lhsT=vals[:, j:j + 1], rhs=sel[:],
                                    start=(k == 0), stop=(k == gw - 1))
                            else:
                                msgs = sb.tile([TILE, TILE], bf16, tag="msgs", bufs=8,
                                               name=f"m{layer}_{st}_{j}")
                                nc.gpsimd.indirect_dma_start(
                                    out=msgs[:], out_offset=None, in_=h_reps[layer - 1][:],
                                    in_offset=bass.IndirectOffsetOnAxis(
                                        ap=idx_t[:, j:j + 1], axis=0))
                                sel = sb.tile([TILE, TILE], bf16, tag="sel", bufs=4,
                                              name=f"s{layer}_{st}_{j}")
                                nc.vector.tensor_tensor(
                                    out=sel[:],
                                    in0=dst_l[:, j:j + 1].to_broadcast([TILE, TILE]),
                                    in1=iota_b_t[:], op=ALU.is_equal)
                                nc.tensor.matmul(
                                    out=agg_ps[:, w * TILE:(w + 1) * TILE],
                                    lhsT=msgs[:], rhs=sel[:],
                                    start=(k == 0), stop=(k == gw - 1))
                            j += 1

                    z_ps = ps.tile([128, 512], f32, tag="z", bufs=2,
                                   name=f"z{layer}_{st}")
                    if layer == 1:
                        mean_t = sb.tile([1, 512], f32, tag="mean1", bufs=2,
                                         name=f"me{layer}_{st}")
                        nc.vector.tensor_tensor(
                            out=mean_t[:], in0=agg_ps[:],
                            in1=cnt_t[0:1, :], op=ALU.mult)
                        xrow = sb.tile([1, 512], f32, tag="xrow", bufs=2,
                                       name=f"xr{layer}_{st}")
                        nc.sync.dma_start(out=xrow[:], in_=x_own[0:1, c0:c0 + 512])
                        nc.tensor.matmul(out=z_ps[:], lhsT=wl_t[1][:], rhs=mean_t[:],
                                         start=True, stop=False)
                        nc.tensor.matmul(out=z_ps[:], lhsT=wr_t[1][:], rhs=xrow[:],
                                         start=False, stop=True)
                    else:
                        mean_t = sb.tile([128, 512], f32, tag="mean", bufs=2,
                                         name=f"me{layer}_{st}")
                        nc.vector.tensor_tensor(
                            out=mean_t[:], in0=agg_ps[:], in1=cnt_t[:], op=ALU.mult)
                        hT_t = sb.tile([128, 512], f32, tag="hTt", bufs=2,
                                       name=f"hp{layer}_{st}")
                        for q in range(TPST):
                            nc.sync.dma_start(
                                out=hT_t[:, q * TILE:(q + 1) * TILE],
                                in_=hT_prev[st * TPST + q])
                        nc.tensor.matmul(out=z_ps[:], lhsT=wl_t[layer][:], rhs=mean_t[:],
                                         start=True, stop=False)
                        nc.tensor.matmul(out=z_ps[:], lhsT=wr_t[layer][:], rhs=hT_t[:],
                                         start=False, stop=True)

                    zT_t = sb.tile([128, 512], f32, tag="zT", bufs=2,
                                   name=f"zt{layer}_{st}")
                    if st == ST_C - 1:
                        mask_t = sb.tile([128, 512], f32, tag="maskt", bufs=1,
                                         name=f"mk{layer}")
                        nc.sync.dma_start(
                            out=mask_t[:],
                            in_=smask[0:1, c0:c0 + 512].to_broadcast([128, 512]))
                        zm = sb.tile([128, 512], f32, tag="zmsk", bufs=1,
                                     name=f"zm{layer}")
                        nc.vector.tensor_scalar(
                            out=zm[:], in0=z_ps[:], scalar1=bias_t[layer][:],
                            scalar2=1.0, op0=ALU.add, op1=ALU.mult)
                        nc.vector.tensor_tensor(
                            out=zm[:], in0=zm[:], in1=mask_t[:], op=ALU.mult)
                        nc.scalar.activation(
                            out=zT_t[:], in_=zm[:], func=AF.Identity,
                            accum_out=stats1[:, st:st + 1])
                    else:
                        nc.scalar.activation(
                            out=zT_t[:], in_=z_ps[:], func=AF.Identity,
                            bias=bias_t[layer][:],
                            accum_out=stats1[:, st:st + 1])
                    zsq = sb.tile([128, 512], f32, tag="zsq", bufs=2,
                                  name=f"zq{layer}_{st}")
                    nc.scalar.activation(
                        out=zsq[:], in_=zT_t[:], func=AF.Square,
                        accum_out=stats2[:, st:st + 1])
                    for q in range(TPST):
                        nc.sync.dma_start(
                            out=zT_d[st * TPST + q],
                            in_=zT_t[:, q * TILE:(q + 1) * TILE])

            # =========================================================
            def norm_affine(layer):
                srow = sb.tile([128, 2], f32, tag="srow", bufs=1, name=f"sr{layer}")
                nc.vector.tensor_reduce(
                    out=srow[:, 0:1], in_=stats1[:], axis=mybir.AxisListType.X,
                    op=mybir.AluOpType.add)
                nc.vector.tensor_reduce(
                    out=srow[:, 1:2], in_=stats2[:], axis=mybir.AxisListType.X,
                    op=mybir.AluOpType.add)
                nc.sync.dma_start(out=ar_in[:], in_=srow[:])
                nc.gpsimd.collective_compute(
                    "AllReduce", mybir.AluOpType.add,
                    ins=[ar_in[:]], outs=[ar_outs[layer][:]],
                    replica_groups=[list(range(NCORE))])
                tots = sb.tile([128, 2], f32, tag="tots", bufs=1, name=f"to{layer}")
                nc.sync.dma_start(out=tots[:], in_=ar_outs[layer][:])
                mu = sb.tile([128, 1], f32, tag="mu", bufs=1, name=f"mu{layer}")
                nc.vector.tensor_tensor(out=mu[:], in0=tots[:, 0:1], in1=invn_t[:],
                                        op=ALU.mult)
                e2 = sb.tile([128, 1], f32, tag="e2", bufs=1, name=f"e2{layer}")
                nc.vector.tensor_tensor(out=e2[:], in0=tots[:, 1:2], in1=invn_t[:],
                                        op=ALU.mult)
                t1 = sb.tile([128, 1], f32, tag="t1", bufs=1, name=f"t1{layer}")
                nc.vector.tensor_tensor(out=t1[:], in0=mu[:], in1=mu[:], op=ALU.mult)
                nc.vector.tensor_tensor(out=t1[:], in0=t1[:], in1=calpha_t[layer][:],
                                        op=ALU.mult)
                var = sb.tile([128, 1], f32, tag="var", bufs=1, name=f"va{layer}")
                nc.vector.tensor_tensor(out=var[:], in0=e2[:], in1=t1[:],
                                        op=ALU.subtract)
                nc.vector.tensor_scalar_add(out=var[:], in0=var[:], scalar1=EPS)
                rinv = sb.tile([128, 1], f32, tag="rinv", bufs=1, name=f"ri{layer}")
                nc.vector.reciprocal(out=rinv[:], in_=var[:])
                scale = sb.tile([128, 1], f32, tag="scale", bufs=1, name=f"sc{layer}")
                nc.scalar.activation(out=scale[:], in_=rinv[:], func=AF.Sqrt)
                nc.vector.tensor_tensor(out=scale[:], in0=scale[:],
                                        in1=gamma_t[layer][:], op=ALU.mult)
                shift = sb.tile([128, 1], f32, tag="shift", bufs=1, name=f"sh{layer}")
                nc.vector.tensor_tensor(out=shift[:], in0=alpha_t[layer][:],
                                        in1=mu[:], op=ALU.mult)
                nc.vector.tensor_tensor(out=shift[:], in0=shift[:], in1=scale[:],
                                        op=ALU.mult)
                nc.vector.tensor_tensor(out=shift[:], in0=beta_t[layer][:],
                                        in1=shift[:], op=ALU.subtract)
                return scale, shift

            # =========================================================
            def pass_b(layer, scale, shift):
                hT_next = hT_b if layer % 2 == 0 else hT_a
                for st in range(ST_C):
                    z_in = sb.tile([128, 512], f32, tag="zin", bufs=3,
                                   name=f"zi{layer}_{st}")
                    for q in range(TPST):
                        nc.sync.dma_start(
                            out=z_in[:, q * TILE:(q + 1) * TILE],
                            in_=zT_d[st * TPST + q])
                    hpT = sb.tile([128, 512], f32, tag="hpT", bufs=3,
                                  name=f"hb{layer}_{st}")
                    nc.scalar.activation(
                        out=hpT[:], in_=z_in[:], func=AF.Relu,
                        bias=shift[:], scale=scale[:])
                    if layer == 5:
                        heads(st, hpT)
                        continue
                    for q in range(TPST):
                        nc.sync.dma_start(
                            out=hT_next[st * TPST + q],
                            in_=hpT[:, q * TILE:(q + 1) * TILE])
                    hb = sb.tile([128, 512], bf16, tag="hbf", bufs=2,
                                 name=f"hf{layer}_{st}")
                    for q in range(TPST):
                        tr_ps = ps.tile([128, 128], f32, tag="misc", bufs=2,
                                        name=f"tp{layer}_{st}_{q}")
                        nc.tensor.transpose(
                            out=tr_ps[:], in_=hpT[:, q * TILE:(q + 1) * TILE],
                            identity=ident_t[:])
                        nc.vector.tensor_copy(
                            out=hb[:, q * TILE:(q + 1) * TILE], in_=tr_ps[:])
                    for q in range(TPST):
                        nc.sync.dma_start(
                            out=ag_in[st * 512 + q * TILE:st * 512 + (q + 1) * TILE, :],
                            in_=hb[:, q * TILE:(q + 1) * TILE])

            # =========================================================
            def heads(st, h5T):
                hsb = {}
                for hi, (_, oc) in enumerate(OUTS):
                    h1 = sb.tile([128, 1024], f32, tag="h1", bufs=2,
                                 name=f"h1_{st}_{hi}")
                    for half, wk, bk in ((0, "w1a", "b1a"), (1, "w1b", "b1b")):
                        p1 = ps.tile([128, 512], f32, tag="misc", bufs=2,
                                     name=f"p1_{st}_{hi}_{half}")
                        nc.tensor.matmul(out=p1[:], lhsT=hw_t[(hi, wk)][:],
                                         rhs=h5T[:], start=True, stop=True)
                        nc.scalar.activation(
                            out=h1[:, half * 512:(half + 1) * 512], in_=p1[:],
                            func=AF.Relu, bias=hw_t[(hi, bk)][:])
                    p2 = ps.tile([128, 512], f32, tag="misc", bufs=2,
                                 name=f"p2_{st}_{hi}")
                    nc.tensor.matmul(out=p2[:], lhsT=hw_t[(hi, "w2a")][:],
                                     rhs=h1[:, 0:512], start=True, stop=False)
                    nc.tensor.matmul(out=p2[:], lhsT=hw_t[(hi, "w2b")][:],
                                     rhs=h1[:, 512:1024], start=False, stop=True)
                    h2 = sb.tile([128, 512], f32, tag="h2", bufs=2,
                                 name=f"h2_{st}_{hi}")
                    nc.scalar.activation(out=h2[:], in_=p2[:], func=AF.Relu,
                                         bias=hw_t[(hi, "b2")][:])
                    p3 = ps.tile([32, 512], f32, tag="psm", bufs=2,
                                 name=f"p3_{st}_{hi}")
                    nc.tensor.matmul(out=p3[:oc, :], lhsT=hw_t[(hi, "w3")][:],
                                     rhs=h2[:], start=True, stop=True)
                    hs = sb.tile([oc, 512], f32, tag=f"hs{hi}", bufs=2,
                                 name=f"hs_{st}_{hi}")
                    nc.scalar.activation(out=hs[:], in_=p3[:oc, :],
                                         func=AF.Identity, bias=hw_t[(hi, "b3")][:])
                    hsb[hi] = hs
                # per-head transpose to node-major, pack columns, 4 chunks
                for q in range(TPST):
                    och = sb.tile([128, OC_TOT], f32, tag="och", bufs=2,
                                  name=f"oc_{st}_{q}")
                    o0 = 0
                    for hi, (_, oc) in enumerate(OUTS):
                        trh = ps.tile([128, 8], f32, tag="psm", bufs=2,
                                      name=f"th_{st}_{q}_{hi}")
                        nc.tensor.transpose(
                            out=trh[:, :oc],
                            in_=hsb[hi][:, q * TILE:(q + 1) * TILE],
                            identity=ident_t[:oc, :oc])
                        nc.vector.tensor_copy(out=och[:, o0:o0 + oc],
                                              in_=trh[:, :oc])
                        o0 += oc
                    r0 = (st * TPST + q) * TILE
                    nc.sync.dma_start(out=out_heads[r0:r0 + TILE, :], in_=och[:])

            # =========================================================
            def allgather(layer):
                nc.gpsimd.collective_compute(
                    "AllGather", mybir.AluOpType.bypass,
                    ins=[ag_in[:]], outs=[h_reps[layer][:]],
                    replica_groups=[list(range(NCORE))])

            for layer in range(1, 6):
                pass_a(layer)
                scale, shift = norm_affine(layer)
                pass_b(layer, scale, shift)
                if layer < 5:
                    allgather(layer)

    nc.compile()
    return nc


# ---------------------------------------------------------------- entry

_cache = {}


def kernel(x, edge_index, params):
    global last_exec_time_ns
    _register_ntff_hook()
    from concourse.bass_utils import run_bass_kernel_spmd

    x = np.asarray(x)
    edge_index = np.asarray(edge_index)
    in_maps, n_st, wsplit, g0_st, G_total, counts, starts = _prep(x, edge_index, params)

    key = ("k", G_total, tuple(n_st))
    if key not in _cache:
        _cache[key] = _build(n_st, wsplit, g0_st, G_total)
    nc = _cache[key]

    import os, tempfile
    trace = os.environ.get("GNN_TRACE", "0") == "1"
    kw = {}
    if trace:
        kw = dict(trace=True, tmpdir=tempfile.mkdtemp(prefix="gnnk_"))
    res = run_bass_kernel_spmd(nc, in_maps, list(range(NCORE)), **kw)
    last_exec_time_ns = res.exec_time_ns

    full = np.concatenate(
        [res.results[c]["out_heads"][: counts[c]] for c in range(NCORE)], axis=0)
    outs = []
    o0 = 0
    for _, oc in OUTS:
        outs.append(np.ascontiguousarray(full[:, o0:o0 + oc], dtype=np.float32))
        o0 += oc
    return tuple(outs)
